# revision 79
# baseline (speedup 1.0000x reference)
"""Causal self-attention (RoPE) Trainium2 Bass kernel, 8-core SPMD.

Sharding: core c = (batch b = c//2, head-group g = c%2). Each core computes
4 of the 8 heads for one batch element end-to-end (QKV projection, RoPE,
causal flash-attention, output projection with its head-group's W_out
columns), producing a partial [T, C] output; the host sums the two
head-group partials per batch.

Device-side layout is "feature-major": activations live as [feature, token]
tiles so every matmul has its contraction on the partition axis with no
on-device transposes. All matmul operands are float32r (full fp32 storage,
reduced-precision multiply, 4x faster than plain fp32 on the PE).

Attention is computed in S^T form: S^T[k, q] = sum_d k_rope[d, k] q_rope[d, q]
with k-token blocks on partitions and q on the free axis. Softmax then needs
only exp (no max subtraction -- inputs are bounded, |S| < ~10) and the
denominator comes free from an extra all-ones column appended to V. Causal
masking zeroes the upper-triangular strips of the diagonal blocks with
affine_select on the (otherwise idle) GpSimd engine.

Engine budget (cost-model): PE ~90us is the binding engine; exp can only run
on ScalarE/ACT (~75us), so everything else is placed to keep PE fed:
 - PSUM evacuations of the qk projection go to DVE (not ACT),
 - the RoPE add and the reciprocal partition-broadcast go to GpSimd,
 - bulk DMA rides the SP queue so GpSimd's masking selects stay low-latency.
The main loop is software-pipelined at ~200-400ns granularity: attention
chunks (S-matmuls -> exp -> AV-matmuls) are emitted with a one-chunk stagger
and projection/output-projection work is woven between chunks as PE filler,
with V/K projection of the last token-slice deferred into the final
iteration where attention alone would leave the PE exp-starved.
"""

import numpy as np
from contextlib import nullcontext as _nullctx

B, T, C = 4, 2048, 512
H_TOT, HD = 8, 64
HL = 4          # heads per core
NCORES = 8
MASK_VAL = -30000.0

_prog_cache = {}
LAST_EXEC_NS = None
LAST_RESULTS = None


def _build_program(t=T):
    import concourse.tile as tile
    from concourse import bacc, mybir

    f32 = mybir.dt.float32
    f32r = mybir.dt.float32r
    Exp = mybir.ActivationFunctionType.Exp

    nt = t // 512      # 512-wide token slices
    nb = t // 128      # 128-wide token blocks

    nc = bacc.Bacc("TRN2", target_bir_lowering=False, debug=False,
                   enable_asserts=False, num_devices=NCORES)

    xT = nc.dram_tensor("xT", [C, t], f32r, kind="ExternalInput").ap()
    wqkT = nc.dram_tensor("wqkT", [C, 512], f32r, kind="ExternalInput").ap()
    wvT = nc.dram_tensor("wvT", [C, 256], f32r, kind="ExternalInput").ap()
    woT = nc.dram_tensor("woT", [256, C], f32r, kind="ExternalInput").ap()
    cos2T = nc.dram_tensor("cos2T", [128, t], f32r, kind="ExternalInput").ap()
    sin2T = nc.dram_tensor("sin2T", [128, t], f32r, kind="ExternalInput").ap()
    r2T = nc.dram_tensor("r2T", [128, 128], f32r, kind="ExternalInput").ap()
    ones4 = nc.dram_tensor("ones4", [128, 4], f32r, kind="ExternalInput").ap()
    y = nc.dram_tensor("y", [t, C], f32, kind="ExternalOutput").ap()

    with tile.TileContext(nc) as tc:
        with tc.tile_pool(name="persist", bufs=1) as pp, \
             tc.tile_pool(name="ptiles", bufs=4) as ppool, \
             tc.tile_pool(name="psum", bufs=1, space="PSUM") as ps:

            # ---- constants & weights ----
            r2_t = pp.tile([128, 128], f32r, tag="r2")
            wq_t = [pp.tile([128, 512], f32r, tag=f"wq{i}", name=f"wq{i}")
                    for i in range(4)]
            wv_t = [pp.tile([128, 256], f32r, tag=f"wv{i}", name=f"wv{i}")
                    for i in range(4)]
            wo_t = [pp.tile([128, 512], f32r, tag=f"wo{i}", name=f"wo{i}")
                    for i in range(2)]
            cos_sl = [pp.tile([128, 512], f32r, tag=f"cos{s_}",
                              name=f"cos{s_}") for s_ in range(nt)]
            sin_sl = [pp.tile([128, 512], f32r, tag=f"sin{s_}",
                              name=f"sin{s_}") for s_ in range(nt)]

            # ---- persistent activations ----
            qk_sl = [[pp.tile([128, 512], f32r, tag=f"qkr{i}_{s_}",
                              name=f"qkr{i}_{s_}") for s_ in range(nt)]
                     for i in range(4)]
            v_aug = [pp.tile([128, 4 * 65], f32r, tag=f"va{i}", name=f"va{i}")
                     for i in range(nb)]

            def load_x(ts):
                # bulk slice loads ride the SP queue (GpSimd queue must stay
                # low-latency for the masking selects / pbcast / rope adds)
                sl = slice(ts * 512, (ts + 1) * 512)
                x_ts = [pp.tile([128, 512], f32r, tag=f"xs{cc}", bufs=2,
                                name=f"xs{cc}") for cc in range(4)]
                for cc in range(4):
                    nc.sync.dma_start(out=x_ts[cc],
                                      in_=xT[cc * 128:(cc + 1) * 128, sl])
                nc.sync.dma_start(out=cos_sl[ts], in_=cos2T[:, sl])
                nc.sync.dma_start(out=sin_sl[ts], in_=sin2T[:, sl])
                return x_ts

            # ---------------- projection pieces (filler-sized) -----------
            CC_ORDER = (0, 1, 3, 2)   # matches prologue DMA arrival order

            def emit_proj_mm(ts, fb, cc, x_ts, psum, first, last):
                nc.tensor.matmul(
                    psum[:, 0:512],
                    wq_t[cc][:, fb * 128:(fb + 1) * 128],
                    x_ts[cc][:],
                    start=first, stop=last)

            def emit_evac(ts, fb, psum, qkp, evac=None):
                # qkp evac on DVE (ACT is reserved for the exp stream; the
                # prologue, before any exp, uses ACT to keep DVE clear)
                (evac or nc.vector.tensor_copy)(out=qkp[:], in_=psum[:, 0:512])

            def emit_rope_rest(ts, fb, qkp):
                # rotate on PE; sin-mul on DVE, cos-mul + add on GpSimd
                rps = ps.tile([128, 512], f32, tag="sd", bufs=2)
                nc.tensor.matmul(rps[:], r2_t[:], qkp[:],
                                 start=True, stop=True)
                tmp = pp.tile([128, 512], f32r, tag="ropetmp", bufs=2)
                nc.vector.tensor_mul(out=tmp[:], in0=rps[:],
                                     in1=sin_sl[ts][:])
                tmp2 = pp.tile([128, 512], f32r, tag="ropetmp2", bufs=2)
                nc.gpsimd.tensor_mul(out=tmp2[:], in0=qkp[:],
                                     in1=cos_sl[ts][:])
                nc.gpsimd.tensor_add(out=qk_sl[fb][ts][:],
                                     in0=tmp[:], in1=tmp2[:])

            def proj_fb_closures(ts, fb, x_ts, tag="pp", bufs=1, evac=None):
                # own single-buffer PSUM tag by default: proj psums are
                # long-lived relative to the chunk-paced attention psums and
                # must not rotate through the same buffers (circular-wait
                # risk). The prologue (no attention in flight yet) borrows
                # the idle "s" slots instead for fb-level overlap.
                # The evac rides with the last matmul and the PE rotate is a
                # separate closure so attention work sits between them in
                # the PE queue, hiding the evacuation latency.
                psum = ps.tile([128, 512], f32, tag=tag, bufs=bufs,
                               name="projps")
                qkp = pp.tile([128, 512], f32r, tag="qkp", bufs=6)

                def mk(i, cc):
                    def f():
                        emit_proj_mm(ts, fb, cc, x_ts, psum,
                                     i == 0, i == 3)
                        if i == 3:
                            emit_evac(ts, fb, psum, qkp, evac=evac)
                    return f
                cls = [mk(i, cc) for i, cc in enumerate(CC_ORDER)]
                cls.append(lambda: emit_rope_rest(ts, fb, qkp))
                return cls

            def emit_vproj(ts, tbl, x_ts, evac=None):
                tb = ts * 4 + tbl
                vpsum = ps.tile([128, 256], f32, tag="sd", bufs=2)
                for cc in range(4):
                    nc.tensor.matmul(
                        vpsum[:],
                        x_ts[cc][:, tbl * 128:(tbl + 1) * 128],
                        wv_t[cc][:],
                        start=(cc == 0), stop=(cc == 3))
                (evac or nc.vector.tensor_copy)(
                    out=v_aug[tb].rearrange(
                        "p (h c) -> p h c", h=4)[:, :, 0:64],
                    in_=vpsum.rearrange("p (h c) -> p h c", h=4))

            def emit_outproj_tbl(qs, tbl, pkq, tail=False):
                # at the tail the attention "s" slots are free -- borrow
                # them so all four output psums can be in flight
                if tail and tbl % 2:
                    ypsum = ps.tile([128, 1024], f32, tag="s", bufs=2,
                                    name="ypsum")
                    ypsum = ypsum[:, 0:512]
                else:
                    ypsum = ps.tile([128, 512], f32, tag="sd", bufs=2,
                                    name="ypsum")
                for fc in range(2):
                    nc.tensor.matmul(
                        ypsum[:],
                        pkq[fc][:, tbl * 128:(tbl + 1) * 128],
                        wo_t[fc][:],
                        start=(fc == 0), stop=(fc == 1))
                ysb = pp.tile([128, 512], f32, tag="ysb", bufs=6)
                if tail and tbl % 2 == 0:
                    # ACT is mostly idle at the tail -- alternate the
                    # evacuations between ACT and DVE so neither serializes
                    nc.scalar.copy(out=ysb[:], in_=ypsum[:])
                else:
                    nc.vector.tensor_copy(out=ysb[:], in_=ypsum[:])
                row = slice((qs * 4 + tbl) * 128, (qs * 4 + tbl + 1) * 128)
                if tail:
                    # spread the final stores across three DMA queues
                    q = (nc.sync, nc.gpsimd, nc.scalar, nc.sync)[tbl]
                    q.dma_start(out=y[row, :], in_=ysb[:])
                else:
                    nc.sync.dma_start(out=y[row, :], in_=ysb[:])

            # ---------------- attention unit, chunked + staggered --------
            def emit_attn(qs, hl, pkq, pull, diag_first=True):
                pairi, half = hl // 2, (hl % 2) * 64
                qh = qk_sl[pairi][qs][half:half + 64, :]

                def kslc(kb):
                    return qk_sl[2 + pairi][kb // 4][
                        half:half + 64, (kb % 4) * 128:(kb % 4 + 1) * 128]

                va = [v_aug[kb][:, hl * 65:(hl + 1) * 65] for kb in range(nb)]
                opsum = ps.tile([65, 512], f32, tag="o", bufs=1)
                pend_av = [None]
                first_av = [True]

                def step(emit_av):
                    # stagger: filler + previous chunk's AV land after this
                    # chunk's S matmuls so the PE never waits on the exp
                    if pend_av[0] is not None:
                        pull()
                        pend_av[0]()
                    pend_av[0] = emit_av

                def start_flag():
                    f = first_av[0]
                    first_av[0] = False
                    return f

                # paired full k-blocks: one exp covers two blocks
                def chunk_pair(kp, last=False):
                    ka, kb_ = 2 * kp, 2 * kp + 1
                    spsum = ps.tile([128, 1024], f32, tag="s", bufs=2)
                    nc.tensor.matmul(
                        spsum[:, 0:512], kslc(ka), qh,
                        start=True, stop=True)
                    nc.tensor.matmul(
                        spsum[:, 512:1024], kslc(kb_), qh,
                        start=True, stop=True)
                    pT = ppool.tile([128, 1024], f32r, tag="pT", bufs=6)
                    nc.scalar.activation(out=pT[:], in_=spsum[:], func=Exp)

                    def av_pair():
                        nc.tensor.matmul(
                            opsum[:], va[ka], pT[:, 0:512],
                            start=start_flag(), stop=False)
                        nc.tensor.matmul(
                            opsum[:], va[kb_], pT[:, 512:1024],
                            start=False, stop=last)
                    step(av_pair)

                # diagonal 512-block: four masked k-blocks packed into two
                # PSUM tiles / two exp instructions, trimmed to the causal
                # columns (widened where float32r would run at 1/4 rate; the
                # widened columns are fully masked and exp'd to exactly 0)
                qsl_t = qk_sl[pairi][qs]

                def chunk_diag_a():
                    spA = ps.tile([128, 1024], f32, tag="s", bufs=2)
                    nc.tensor.matmul(
                        spA[:, 0:512], kslc(4 * qs),
                        qsl_t[half:half + 64, 0:512], start=True, stop=True)
                    nc.tensor.matmul(
                        spA[:, 512:896], kslc(4 * qs + 1),
                        qsl_t[half:half + 64, 128:512], start=True, stop=True)
                    pTa = ppool.tile([128, 1024], f32r, tag="pT", bufs=6)
                    nc.scalar.activation(out=pTa[:, 0:896], in_=spA[:, 0:896],
                                         func=Exp)
                    # masking selects are on the AV critical path -- keep
                    # them ahead of concurrent filler work in the Pool queue
                    with tc.high_priority(offset=60):
                        for c0 in (0, 512):
                            nc.gpsimd.affine_select(
                                out=pTa[:, c0:c0 + 128],
                                in_=pTa[:, c0:c0 + 128],
                                compare_op=mybir.AluOpType.is_ge, fill=0.0,
                                base=0, channel_multiplier=-1,
                                pattern=[[1, 128]])

                    def av_diag_a():
                        nc.tensor.matmul(
                            opsum[:, 0:512], va[4 * qs], pTa[:, 0:512],
                            start=start_flag(), stop=False)
                        nc.tensor.matmul(
                            opsum[:, 128:512], va[4 * qs + 1],
                            pTa[:, 512:896],
                            start=False, stop=False)
                    step(av_diag_a)

                def chunk_diag_b(last=False):
                    spB = ps.tile([128, 1024], f32, tag="s", bufs=2)
                    nc.tensor.matmul(
                        spB[:, 0:256], kslc(4 * qs + 2),
                        qsl_t[half:half + 64, 256:512], start=True, stop=True)
                    nc.tensor.matmul(
                        spB[:, 256:512], kslc(4 * qs + 3),
                        qsl_t[half:half + 64, 256:512], start=True, stop=True)
                    pTb = ppool.tile([128, 512], f32r, tag="pTd", bufs=6)
                    nc.scalar.activation(out=pTb[:], in_=spB[:, 0:512],
                                         func=Exp)
                    with tc.high_priority(offset=60):
                        nc.gpsimd.affine_select(
                            out=pTb[:, 0:128], in_=pTb[:, 0:128],
                            compare_op=mybir.AluOpType.is_ge, fill=0.0,
                            base=0, channel_multiplier=-1, pattern=[[1, 128]])
                        nc.gpsimd.affine_select(
                            out=pTb[:, 256:512], in_=pTb[:, 256:512],
                            compare_op=mybir.AluOpType.is_ge, fill=0.0,
                            base=-128, channel_multiplier=-1,
                            pattern=[[1, 256]])

                    def av_diag_b():
                        nc.tensor.matmul(
                            opsum[:, 256:512], va[4 * qs + 2], pTb[:, 0:256],
                            start=False, stop=False)
                        nc.tensor.matmul(
                            opsum[:, 256:512], va[4 * qs + 3],
                            pTb[:, 256:512],
                            start=False, stop=last)
                    step(av_diag_b)

                # diag-first: the unit then ENDS on a plain pair chunk whose
                # exp->AV chain is short, so the unit seam (and the kernel
                # tail for the very last unit) isn't gated by the long
                # exp->select->AV diagonal chain. Requires K/V of the diag
                # blocks at unit start (true except the final iteration's
                # first unit, whose V(3)/K3 arrive as fillers mid-unit).
                if diag_first and qs > 0:
                    chunk_diag_a()
                    chunk_diag_b()
                    for kp in range(2 * qs):
                        chunk_pair(kp, last=(kp == 2 * qs - 1))
                else:
                    for kp in range(2 * qs):
                        chunk_pair(kp)
                    chunk_diag_a()
                    chunk_diag_b(last=True)
                pull()
                pend_av[0]()

                # normalize: reciprocal straight from the PSUM denominator
                # row, partition-broadcast on GpSimd, multiply on DVE. The
                # very last unit multiplies straight out of PSUM (no
                # evacuation on its critical path).
                last_unit = (qs == nt - 1 and hl == HL - 1)
                recip_t = pp.tile([1, 512], f32r, tag="recip", bufs=3)
                with nc.allow_low_precision(reason="f32 recip"):
                    with tc.high_priority() if last_unit else _nullctx():
                        nc.vector.reciprocal(out=recip_t[:],
                                             in_=opsum[64:65, :])
                bc = pp.tile([64, 512], f32r, tag="bc", bufs=3)
                with tc.high_priority() if last_unit else _nullctx():
                    nc.gpsimd.partition_broadcast(bc[:], recip_t[:],
                                                  channels=64)
                if last_unit:
                    # the whole tail hangs off this multiply: schedule it
                    # ahead of any concurrent DVE filler evacuations, split
                    # in halves so the first token blocks start early
                    with tc.high_priority():
                        for c0 in (0, 256):
                            nc.vector.tensor_mul(
                                out=pkq[pairi][half:half + 64, c0:c0 + 256],
                                in0=opsum[0:64, c0:c0 + 256],
                                in1=bc[:, c0:c0 + 256])
                else:
                    ot = pp.tile([65, 512], f32r, tag="ot", bufs=4)
                    nc.vector.tensor_copy(out=ot[0:64, :],
                                          in_=opsum[0:64, :])
                    if half == 0:
                        # GpSimd requires matching partition ranges on all
                        # operands, so only even head-slots qualify; this
                        # halves the normalize load on DVE
                        nc.gpsimd.tensor_mul(
                            out=pkq[pairi][0:64, :],
                            in0=ot[0:64, :], in1=bc[:])
                    else:
                        nc.vector.tensor_mul(
                            out=pkq[pairi][half:half + 64, :],
                            in0=ot[0:64, :], in1=bc[:])

            # ---------------- prologue: spread initial loads -------------
            sl0 = slice(0, 512)
            x_cur = [pp.tile([128, 512], f32r, tag=f"xs{cc}", bufs=2,
                             name=f"xs{cc}") for cc in range(4)]
            # first-wave transfers split across the 3 DMA-capable queues
            # (Pool / SP / ACT) so the first projection matmuls can start
            # ~2.8us in. The ACT queue head carries the hoisted 1.3us
            # activation-table load, so ACT only gets late-needed items.
            nc.gpsimd.dma_start(out=x_cur[0], in_=xT[0:128, sl0])
            nc.gpsimd.dma_start(out=wq_t[1], in_=wqkT[128:256, :])
            nc.gpsimd.dma_start(out=x_cur[3], in_=xT[384:512, sl0])
            nc.gpsimd.dma_start(out=cos_sl[0], in_=cos2T[:, 0:512])
            nc.gpsimd.dma_start(out=wv_t[0], in_=wvT[0:128, :])
            nc.sync.dma_start(out=wq_t[0], in_=wqkT[0:128, :])
            nc.sync.dma_start(out=x_cur[1], in_=xT[128:256, sl0])
            nc.sync.dma_start(out=x_cur[2], in_=xT[256:384, sl0])
            nc.sync.dma_start(out=wq_t[3], in_=wqkT[384:512, :])
            nc.sync.dma_start(out=sin_sl[0], in_=sin2T[:, 0:512])
            nc.sync.dma_start(out=r2_t, in_=r2T)
            nc.sync.dma_start(out=wv_t[1], in_=wvT[128:256, :])
            nc.sync.dma_start(out=wv_t[3], in_=wvT[384:512, :])
            ones_sb = pp.tile([128, 4], f32r, tag="ones_sb")
            nc.sync.dma_start(out=ones_sb, in_=ones4)
            nc.scalar.dma_start(out=wq_t[2], in_=wqkT[256:384, :])
            # proj(0): only fb0 (q pair0) and fb2 (k pair0) before attention
            # starts -- units (0, hl=0/1) depend just on these + v_aug[0:3].
            # fb1/fb3 of slice 0 flow in as the first iteration-0 fillers.
            p0 = proj_fb_closures(0, 0, x_cur, tag="s", bufs=2,
                                  evac=nc.scalar.copy)
            p2 = proj_fb_closures(0, 2, x_cur, tag="s", bufs=2,
                                  evac=nc.scalar.copy)
            for cl in p0[:4]:
                cl()
            for cl in p2[:4]:
                cl()
            # wv2 rides the ACT queue after the prologue evacuations so it
            # doesn't delay them
            nc.scalar.dma_start(out=wv_t[2], in_=wvT[256:384, :])
            # V-projections interleave with the rope tails so the PE rotate
            # never head-of-line blocks on its evacuation
            p0[4]()   # rope fb0
            emit_vproj(0, 0, x_cur)
            emit_vproj(0, 1, x_cur)
            p2[4]()   # rope fb2
            emit_vproj(0, 2, x_cur)
            emit_vproj(0, 3, x_cur)
            # ones columns of the persistent v_aug tiles, written once,
            # after the slice-0 V copies so those aren't queued behind them
            # (memset doesn't survive neuronxcc codegen: DMA + strided copy)
            for tb in range(nb):
                nc.vector.tensor_copy(
                    out=v_aug[tb].rearrange(
                        "p (h c) -> p h c", h=4)[:, :, 64:65],
                    in_=ones_sb.rearrange("p (h c) -> p h c", c=1))
            prologue_fill = (proj_fb_closures(0, 1, x_cur)
                             + proj_fb_closures(0, 3, x_cur))

            # ---------------- software-pipelined main loop ---------------
            pkq_prev = None
            pkq_by_ts = {}
            x_by_ts = {0: x_cur}
            for ts in range(nt):
                if ts + 1 < nt:
                    x_nxt = load_x(ts + 1)
                    x_by_ts[ts + 1] = x_nxt
                    if ts == 0:
                        for i in range(2):
                            nc.sync.dma_start(
                                out=wo_t[i], in_=woT[i * 128:(i + 1) * 128, :])
                else:
                    x_nxt = None
                pkq = [pp.tile([128, 512], f32r, tag=f"pkq{i}", bufs=4,
                               name=f"pkq{i}") for i in range(2)]
                pkq_by_ts[ts] = pkq

                # filler queue for this iteration (dependency-safe order).
                # The final iteration's attention alone is exp-bound (the
                # PE would starve behind the ACT exp stream), so every
                # deferrable piece of PE work is parked there: the last
                # slice's V projection and K-pair-1 projection, and the
                # output projections of slices 0 and 2. outproj(1) fills
                # iteration 2, which is roughly exp/PE-balanced.
                def weave(fb_lists, tail_items):
                    # lay out projection fillers so each fb's PE rotate
                    # (which head-of-line blocks the PE queue on its psum
                    # evacuation) lands a full block of other work after
                    # that evacuation was issued
                    out, prev_rest = [], None
                    for L in fb_lists:
                        out.extend(L[:4])
                        if prev_rest is not None:
                            out.append(prev_rest)
                        prev_rest = L[4]
                    extras = list(tail_items)
                    if prev_rest is not None:
                        if extras:
                            out.append(extras.pop(0))
                        out.append(prev_rest)
                    out.extend(extras)
                    return out

                fillers = []
                if x_nxt is not None:
                    nxt = ts + 1
                    last = (nxt == nt - 1)
                    # early iterations evacuate on ACT (exp stream has
                    # slack there and DVE is the scarce engine); late
                    # iterations on DVE (ACT is exp-saturated)
                    ev = nc.scalar.copy if ts < 2 else None
                    fbs = (0, 2, 1) if last else (0, 2, 1, 3)
                    fb_lists = ([] if ts != 0 else
                                [prologue_fill[0:5], prologue_fill[5:10]])
                    fb_lists += [proj_fb_closures(nxt, fb, x_nxt, evac=ev)
                                 for fb in fbs]
                    vps = [] if last else [
                        (lambda nxt=nxt, tbl=tbl, x=x_nxt:
                         emit_vproj(nxt, tbl, x)) for tbl in range(4)]
                    fillers = weave(fb_lists, vps)
                else:
                    # final iteration: V(3) first (needed by the first
                    # unit's trailing diagonal), then K3 pair 1, then the
                    # deferred output projections of slices 1, 0, 2
                    x_fin = x_by_ts[ts]
                    fillers = [
                        (lambda tbl=tbl, x=x_fin: emit_vproj(ts, tbl, x))
                        for tbl in range(4)]
                    ops = []
                    for qs_ in (1, 0, 2):
                        for tbl in range(4):
                            ops.append(
                                lambda qs=qs_, tbl=tbl, p=pkq_by_ts[qs_]:
                                emit_outproj_tbl(qs, tbl, p))
                    fillers += weave([proj_fb_closures(ts, 3, x_fin)], ops)

                # dynamic pacing: spread the fillers evenly over the
                # iteration's pull points so late units still have PE work
                # and early fillers (next slice's rope chains) land well
                # before the next iteration's attention needs them
                n_fill = len(fillers)
                n_pulls = 4 * (2 * ts + 2)
                if ts == nt - 1:
                    # front-load the final iteration's fillers: any excess
                    # PE work woven into the last unit pushes its closing
                    # AVs -- and with them the whole kernel tail -- later
                    n_pulls -= 6
                state = [0, 0]   # executed, pulls seen

                def pull():
                    state[1] += 1
                    target = min(n_fill,
                                 -(-n_fill * state[1] // n_pulls))
                    while state[0] < target:
                        fillers[state[0]]()
                        state[0] += 1

                for hl in range(HL):
                    emit_attn(ts, hl, pkq, pull,
                              diag_first=not (ts == nt - 1 and hl == 0))
                # drain leftover fillers before the next iteration
                while state[0] < n_fill:
                    fillers[state[0]]()
                    state[0] += 1
                pkq_prev = pkq
            for tbl in range(4):
                emit_outproj_tbl(nt - 1, tbl, pkq_prev, tail=True)

    nc.compile()
    return nc


def _rot_mats():
    r = np.zeros((64, 64), np.float32)
    r[np.arange(32), np.arange(32) + 32] = -1.0
    r[np.arange(32, 64), np.arange(32)] = 1.0
    r2 = np.zeros((128, 128), np.float32)
    r2[:64, :64] = r
    r2[64:, 64:] = r
    return np.ascontiguousarray(r2.T)


def _preprocess(x, cos, sin, W_qkv, W_out, t=T):
    r2T = _rot_mats()
    cosT = np.ascontiguousarray(cos[:t].T.astype(np.float32))
    sinT = np.ascontiguousarray(sin[:t].T.astype(np.float32))
    cos2 = np.concatenate([cosT, cosT], 0)
    sin2 = np.concatenate([sinT, sinT], 0)

    in_maps = []
    for c in range(NCORES):
        b, g = divmod(c, 2)
        wq = W_qkv[g * 256:(g + 1) * 256] * 0.125
        wk = W_qkv[512 + g * 256:512 + (g + 1) * 256]
        wv = W_qkv[1024 + g * 256:1024 + (g + 1) * 256]
        in_maps.append({
            "xT": np.ascontiguousarray(x[b, :t].T.astype(np.float32)),
            "wqkT": np.ascontiguousarray(
                np.concatenate([wq, wk], 0).T.astype(np.float32)),
            "wvT": np.ascontiguousarray(wv.T.astype(np.float32)),
            "woT": np.ascontiguousarray(
                W_out.T[g * 256:(g + 1) * 256].astype(np.float32)),
            "cos2T": cos2, "sin2T": sin2, "r2T": r2T,
            "ones4": np.ones((128, 4), np.float32),
        })
    return in_maps


def kernel(x, cos, sin, W_qkv, W_out, _trace=False):
    global LAST_EXEC_NS, LAST_RESULTS
    from concourse.bass_utils import run_bass_kernel_spmd

    x = np.asarray(x); cos = np.asarray(cos); sin = np.asarray(sin)
    W_qkv = np.asarray(W_qkv); W_out = np.asarray(W_out)

    if T not in _prog_cache:
        _prog_cache[T] = _build_program(T)
    nc = _prog_cache[T]

    in_maps = _preprocess(x, cos, sin, W_qkv, W_out)
    try:
        res = run_bass_kernel_spmd(nc, in_maps, list(range(NCORES)),
                                   trace=_trace)
    except ModuleNotFoundError:
        # NTFF profiling hooks unavailable under this axon build
        res = run_bass_kernel_spmd(nc, in_maps, list(range(NCORES)),
                                   trace=False)
    LAST_EXEC_NS = res.exec_time_ns
    LAST_RESULTS = res
    out = np.empty((B, T, C), np.float32)
    for b in range(B):
        out[b] = res.results[2 * b]["y"] + res.results[2 * b + 1]["y"]
    return out


# revision 88
# speedup vs baseline: 1.0010x; 1.0010x over previous
"""Causal self-attention (RoPE) Trainium2 Bass kernel, 8-core SPMD.

Sharding: core c = (batch b = c//2, head-group g = c%2). Each core computes
4 of the 8 heads for one batch element end-to-end (QKV projection, RoPE,
causal flash-attention, output projection with its head-group's W_out
columns), producing a partial [T, C] output; the host sums the two
head-group partials per batch.

Device-side layout is "feature-major": activations live as [feature, token]
tiles so every matmul has its contraction on the partition axis with no
on-device transposes. All matmul operands are float32r (full fp32 storage,
reduced-precision multiply, 4x faster than plain fp32 on the PE).

Attention is computed in S^T form: S^T[k, q] = sum_d k_rope[d, k] q_rope[d, q]
with k-token blocks on partitions and q on the free axis. Softmax then needs
only exp (no max subtraction -- inputs are bounded, |S| < ~10) and the
denominator comes free from an extra all-ones column appended to V. Causal
masking zeroes the upper-triangular strips of the diagonal blocks with
affine_select on the (otherwise idle) GpSimd engine.

Engine budget (cost-model): PE ~90us is the binding engine; exp can only run
on ScalarE/ACT (~75us), so everything else is placed to keep PE fed:
 - PSUM evacuations of the qk projection go to DVE (not ACT),
 - the RoPE add and the reciprocal partition-broadcast go to GpSimd,
 - bulk DMA rides the SP queue so GpSimd's masking selects stay low-latency.
The main loop is software-pipelined at ~200-400ns granularity: attention
chunks (S-matmuls -> exp -> AV-matmuls) are emitted with a one-chunk stagger
and projection/output-projection work is woven between chunks as PE filler,
with V/K projection of the last token-slice deferred into the final
iteration where attention alone would leave the PE exp-starved.
"""

import numpy as np
from contextlib import nullcontext as _nullctx

B, T, C = 4, 2048, 512
H_TOT, HD = 8, 64
HL = 4          # heads per core
NCORES = 8
MASK_VAL = -30000.0

_prog_cache = {}
LAST_EXEC_NS = None
LAST_RESULTS = None


def _build_program(t=T):
    import concourse.tile as tile
    from concourse import bacc, mybir

    f32 = mybir.dt.float32
    f32r = mybir.dt.float32r
    Exp = mybir.ActivationFunctionType.Exp

    nt = t // 512      # 512-wide token slices
    nb = t // 128      # 128-wide token blocks

    nc = bacc.Bacc("TRN2", target_bir_lowering=False, debug=False,
                   enable_asserts=False, num_devices=NCORES)

    xT = nc.dram_tensor("xT", [C, t], f32r, kind="ExternalInput").ap()
    wqkT = nc.dram_tensor("wqkT", [C, 512], f32r, kind="ExternalInput").ap()
    wvT = nc.dram_tensor("wvT", [C, 256], f32r, kind="ExternalInput").ap()
    woT = nc.dram_tensor("woT", [256, C], f32r, kind="ExternalInput").ap()
    cos2T = nc.dram_tensor("cos2T", [128, t], f32r, kind="ExternalInput").ap()
    sin2T = nc.dram_tensor("sin2T", [128, t], f32r, kind="ExternalInput").ap()
    r2T = nc.dram_tensor("r2T", [128, 128], f32r, kind="ExternalInput").ap()
    ones4 = nc.dram_tensor("ones4", [128, 4], f32r, kind="ExternalInput").ap()
    y = nc.dram_tensor("y", [t, C], f32, kind="ExternalOutput").ap()

    with tile.TileContext(nc) as tc:
        with tc.tile_pool(name="persist", bufs=1) as pp, \
             tc.tile_pool(name="ptiles", bufs=4) as ppool, \
             tc.tile_pool(name="psum", bufs=1, space="PSUM") as ps:

            # ---- constants & weights ----
            r2_t = pp.tile([128, 128], f32r, tag="r2")
            wq_t = [pp.tile([128, 512], f32r, tag=f"wq{i}", name=f"wq{i}")
                    for i in range(4)]
            wv_t = [pp.tile([128, 256], f32r, tag=f"wv{i}", name=f"wv{i}")
                    for i in range(4)]
            wo_t = [pp.tile([128, 512], f32r, tag=f"wo{i}", name=f"wo{i}")
                    for i in range(2)]
            cos_sl = [pp.tile([128, 512], f32r, tag=f"cos{s_}",
                              name=f"cos{s_}") for s_ in range(nt)]
            sin_sl = [pp.tile([128, 512], f32r, tag=f"sin{s_}",
                              name=f"sin{s_}") for s_ in range(nt)]

            # ---- persistent activations ----
            qk_sl = [[pp.tile([128, 512], f32r, tag=f"qkr{i}_{s_}",
                              name=f"qkr{i}_{s_}") for s_ in range(nt)]
                     for i in range(4)]
            v_aug = [pp.tile([128, 4 * 65], f32r, tag=f"va{i}", name=f"va{i}")
                     for i in range(nb)]

            def load_x(ts, spread=False):
                # bulk slice loads ride the SP queue (GpSimd queue must stay
                # low-latency for the masking selects / pbcast / rope adds).
                # Slice 1 is needed while the SP queue still drains the
                # prologue, so its loads spread across three queues.
                sl = slice(ts * 512, (ts + 1) * 512)
                x_ts = [pp.tile([128, 512], f32r, tag=f"xs{cc}", bufs=2,
                                name=f"xs{cc}") for cc in range(4)]
                qs_ = ((nc.sync, nc.sync, nc.gpsimd, nc.gpsimd)
                       if spread else (nc.sync,) * 4)
                for cc in range(4):
                    qs_[cc].dma_start(out=x_ts[cc],
                                      in_=xT[cc * 128:(cc + 1) * 128, sl])
                (nc.gpsimd if spread else nc.sync).dma_start(
                    out=cos_sl[ts], in_=cos2T[:, sl])
                (nc.scalar if spread else nc.sync).dma_start(
                    out=sin_sl[ts], in_=sin2T[:, sl])
                return x_ts

            # ---------------- projection pieces (filler-sized) -----------
            CC_ORDER = (0, 1, 2, 3)   # matches prologue DMA arrival order

            def emit_proj_mm(ts, fb, cc, x_ts, psum, first, last):
                nc.tensor.matmul(
                    psum[:, 0:512],
                    wq_t[cc][:, fb * 128:(fb + 1) * 128],
                    x_ts[cc][:],
                    start=first, stop=last)

            def emit_evac(ts, fb, psum, qkp, evac=None):
                # qkp evac on DVE (ACT is reserved for the exp stream; the
                # prologue, before any exp, uses ACT to keep DVE clear)
                (evac or nc.vector.tensor_copy)(out=qkp[:], in_=psum[:, 0:512])

            def emit_rope_rest(ts, fb, qkp):
                # rotate on PE; sin-mul on DVE, cos-mul + add on GpSimd
                rps = ps.tile([128, 512], f32, tag="sd", bufs=2)
                nc.tensor.matmul(rps[:], r2_t[:], qkp[:],
                                 start=True, stop=True)
                tmp = pp.tile([128, 512], f32r, tag="ropetmp", bufs=2)
                nc.vector.tensor_mul(out=tmp[:], in0=rps[:],
                                     in1=sin_sl[ts][:])
                tmp2 = pp.tile([128, 512], f32r, tag="ropetmp2", bufs=2)
                nc.gpsimd.tensor_mul(out=tmp2[:], in0=qkp[:],
                                     in1=cos_sl[ts][:])
                nc.gpsimd.tensor_add(out=qk_sl[fb][ts][:],
                                     in0=tmp[:], in1=tmp2[:])

            def proj_fb_closures(ts, fb, x_ts, tag="pp", bufs=1, evac=None):
                # own single-buffer PSUM tag by default: proj psums are
                # long-lived relative to the chunk-paced attention psums and
                # must not rotate through the same buffers (circular-wait
                # risk). The prologue (no attention in flight yet) borrows
                # the idle "s" slots instead for fb-level overlap.
                # The evac rides with the last matmul and the PE rotate is a
                # separate closure so attention work sits between them in
                # the PE queue, hiding the evacuation latency.
                psum = ps.tile([128, 512], f32, tag=tag, bufs=bufs,
                               name="projps")
                qkp = pp.tile([128, 512], f32r, tag="qkp", bufs=4)

                def mk(i, cc):
                    def f():
                        emit_proj_mm(ts, fb, cc, x_ts, psum,
                                     i == 0, i == 3)
                        if i == 3:
                            emit_evac(ts, fb, psum, qkp, evac=evac)
                    return f
                cls = [mk(i, cc) for i, cc in enumerate(CC_ORDER)]
                cls.append(lambda: emit_rope_rest(ts, fb, qkp))
                return cls

            def emit_vproj(ts, tbl, x_ts, evac=None):
                tb = ts * 4 + tbl
                vpsum = ps.tile([128, 256], f32, tag="sd", bufs=2)
                for cc in range(4):
                    nc.tensor.matmul(
                        vpsum[:],
                        x_ts[cc][:, tbl * 128:(tbl + 1) * 128],
                        wv_t[cc][:],
                        start=(cc == 0), stop=(cc == 3))
                (evac or nc.vector.tensor_copy)(
                    out=v_aug[tb].rearrange(
                        "p (h c) -> p h c", h=4)[:, :, 0:64],
                    in_=vpsum.rearrange("p (h c) -> p h c", h=4))

            def emit_outproj_tbl(qs, tbl, pkq, tail=False):
                # at the tail the attention "s" slots are free -- borrow
                # them so all four output psums can be in flight
                if tail and tbl % 2:
                    ypsum = ps.tile([128, 1024], f32, tag="s", bufs=2,
                                    name="ypsum")
                    ypsum = ypsum[:, 0:512]
                else:
                    ypsum = ps.tile([128, 512], f32, tag="sd", bufs=2,
                                    name="ypsum")
                for fc in range(2):
                    nc.tensor.matmul(
                        ypsum[:],
                        pkq[fc][:, tbl * 128:(tbl + 1) * 128],
                        wo_t[fc][:],
                        start=(fc == 0), stop=(fc == 1))
                ysb = pp.tile([128, 512], f32, tag="ysb", bufs=5)
                if tail and tbl % 2 == 1:
                    # ACT is mostly idle at the tail -- alternate the
                    # evacuations between ACT and DVE so neither serializes
                    nc.scalar.copy(out=ysb[:], in_=ypsum[:])
                else:
                    nc.vector.tensor_copy(out=ysb[:], in_=ypsum[:])
                row = slice((qs * 4 + tbl) * 128, (qs * 4 + tbl + 1) * 128)
                if tail:
                    # spread the final stores across three DMA queues
                    q = (nc.sync, nc.gpsimd, nc.scalar, nc.sync)[tbl]
                    q.dma_start(out=y[row, :], in_=ysb[:])
                else:
                    nc.sync.dma_start(out=y[row, :], in_=ysb[:])

            # ---------------- attention unit, chunked + staggered --------
            def emit_attn(qs, hl, pkq, pull, diag_first=True):
                pairi, half = hl // 2, (hl % 2) * 64
                qh = qk_sl[pairi][qs][half:half + 64, :]

                def kslc(kb):
                    return qk_sl[2 + pairi][kb // 4][
                        half:half + 64, (kb % 4) * 128:(kb % 4 + 1) * 128]

                va = [v_aug[kb][:, hl * 65:(hl + 1) * 65] for kb in range(nb)]
                opsum = ps.tile([65, 512], f32, tag="o", bufs=1)
                pend_av = [None]
                first_av = [True]

                def step(emit_av):
                    # stagger: filler + previous chunk's AV land after this
                    # chunk's S matmuls so the PE never waits on the exp
                    if pend_av[0] is not None:
                        pull()
                        pend_av[0]()
                    pend_av[0] = emit_av

                def start_flag():
                    f = first_av[0]
                    first_av[0] = False
                    return f

                # paired full k-blocks: one exp covers two blocks
                def chunk_pair(kp, last=False):
                    ka, kb_ = 2 * kp, 2 * kp + 1
                    spsum = ps.tile([128, 1024], f32, tag="s", bufs=2)
                    nc.tensor.matmul(
                        spsum[:, 0:512], kslc(ka), qh,
                        start=True, stop=True)
                    nc.tensor.matmul(
                        spsum[:, 512:1024], kslc(kb_), qh,
                        start=True, stop=True)
                    pT = ppool.tile([128, 1024], f32r, tag="pT", bufs=8)
                    nc.scalar.activation(out=pT[:], in_=spsum[:], func=Exp)

                    def av_pair():
                        nc.tensor.matmul(
                            opsum[:], va[ka], pT[:, 0:512],
                            start=start_flag(), stop=False)
                        nc.tensor.matmul(
                            opsum[:], va[kb_], pT[:, 512:1024],
                            start=False, stop=last)
                    step(av_pair)

                # diagonal 512-block: four masked k-blocks packed into two
                # PSUM tiles / two exp instructions, trimmed to the causal
                # columns (widened where float32r would run at 1/4 rate; the
                # widened columns are fully masked and exp'd to exactly 0)
                qsl_t = qk_sl[pairi][qs]

                def chunk_diag_a():
                    spA = ps.tile([128, 1024], f32, tag="s", bufs=2)
                    nc.tensor.matmul(
                        spA[:, 0:512], kslc(4 * qs),
                        qsl_t[half:half + 64, 0:512], start=True, stop=True)
                    nc.tensor.matmul(
                        spA[:, 512:896], kslc(4 * qs + 1),
                        qsl_t[half:half + 64, 128:512], start=True, stop=True)
                    pTa = ppool.tile([128, 1024], f32r, tag="pT", bufs=8)
                    nc.scalar.activation(out=pTa[:, 0:896], in_=spA[:, 0:896],
                                         func=Exp)
                    # masking selects are on the AV critical path -- keep
                    # them ahead of concurrent filler work in the Pool queue
                    with tc.high_priority(offset=60):
                        for c0 in (0, 512):
                            nc.gpsimd.affine_select(
                                out=pTa[:, c0:c0 + 128],
                                in_=pTa[:, c0:c0 + 128],
                                compare_op=mybir.AluOpType.is_ge, fill=0.0,
                                base=0, channel_multiplier=-1,
                                pattern=[[1, 128]])

                    def av_diag_a():
                        nc.tensor.matmul(
                            opsum[:, 0:512], va[4 * qs], pTa[:, 0:512],
                            start=start_flag(), stop=False)
                        nc.tensor.matmul(
                            opsum[:, 128:512], va[4 * qs + 1],
                            pTa[:, 512:896],
                            start=False, stop=False)
                    step(av_diag_a)

                def chunk_diag_b(last=False):
                    spB = ps.tile([128, 1024], f32, tag="s", bufs=2)
                    nc.tensor.matmul(
                        spB[:, 0:256], kslc(4 * qs + 2),
                        qsl_t[half:half + 64, 256:512], start=True, stop=True)
                    nc.tensor.matmul(
                        spB[:, 256:512], kslc(4 * qs + 3),
                        qsl_t[half:half + 64, 256:512], start=True, stop=True)
                    pTb = ppool.tile([128, 512], f32r, tag="pTd", bufs=6)
                    nc.scalar.activation(out=pTb[:], in_=spB[:, 0:512],
                                         func=Exp)
                    with tc.high_priority(offset=60):
                        nc.gpsimd.affine_select(
                            out=pTb[:, 0:128], in_=pTb[:, 0:128],
                            compare_op=mybir.AluOpType.is_ge, fill=0.0,
                            base=0, channel_multiplier=-1, pattern=[[1, 128]])
                        nc.gpsimd.affine_select(
                            out=pTb[:, 256:512], in_=pTb[:, 256:512],
                            compare_op=mybir.AluOpType.is_ge, fill=0.0,
                            base=-128, channel_multiplier=-1,
                            pattern=[[1, 256]])

                    def av_diag_b():
                        nc.tensor.matmul(
                            opsum[:, 256:512], va[4 * qs + 2], pTb[:, 0:256],
                            start=False, stop=False)
                        nc.tensor.matmul(
                            opsum[:, 256:512], va[4 * qs + 3],
                            pTb[:, 256:512],
                            start=False, stop=last)
                    step(av_diag_b)

                # diag-first: the unit then ENDS on a plain pair chunk whose
                # exp->AV chain is short, so the unit seam (and the kernel
                # tail for the very last unit) isn't gated by the long
                # exp->select->AV diagonal chain. Requires K/V of the diag
                # blocks at unit start (true except the final iteration's
                # first unit, whose V(3)/K3 arrive as fillers mid-unit).
                if diag_first and qs > 0:
                    chunk_diag_a()
                    chunk_diag_b()
                    for kp in range(2 * qs):
                        chunk_pair(kp, last=(kp == 2 * qs - 1))
                else:
                    for kp in range(2 * qs):
                        chunk_pair(kp)
                    chunk_diag_a()
                    chunk_diag_b(last=True)
                pull()
                pend_av[0]()

                # normalize: reciprocal straight from the PSUM denominator
                # row, partition-broadcast on GpSimd, multiply on DVE. The
                # very last unit multiplies straight out of PSUM (no
                # evacuation on its critical path).
                last_unit = (qs == nt - 1 and hl == HL - 1)
                recip_t = pp.tile([1, 512], f32r, tag="recip", bufs=3)
                with nc.allow_low_precision(reason="f32 recip"):
                    with tc.high_priority() if last_unit else _nullctx():
                        nc.vector.reciprocal(out=recip_t[:],
                                             in_=opsum[64:65, :])
                bc = pp.tile([64, 512], f32r, tag="bc", bufs=3)
                with tc.high_priority() if last_unit else _nullctx():
                    nc.gpsimd.partition_broadcast(bc[:], recip_t[:],
                                                  channels=64)
                if last_unit:
                    # the whole tail hangs off this multiply: schedule it
                    # ahead of any concurrent DVE filler evacuations, split
                    # in halves so the first token blocks start early
                    with tc.high_priority():
                        for c0 in (0, 256):
                            nc.vector.tensor_mul(
                                out=pkq[pairi][half:half + 64, c0:c0 + 256],
                                in0=opsum[0:64, c0:c0 + 256],
                                in1=bc[:, c0:c0 + 256])
                else:
                    ot = pp.tile([65, 512], f32r, tag="ot", bufs=3)
                    nc.vector.tensor_copy(out=ot[0:64, :],
                                          in_=opsum[0:64, :])
                    if half == 0:
                        # GpSimd requires matching partition ranges on all
                        # operands, so only even head-slots qualify; this
                        # halves the normalize load on DVE
                        nc.gpsimd.tensor_mul(
                            out=pkq[pairi][0:64, :],
                            in0=ot[0:64, :], in1=bc[:])
                    else:
                        nc.vector.tensor_mul(
                            out=pkq[pairi][half:half + 64, :],
                            in0=ot[0:64, :], in1=bc[:])

            # ---------------- prologue: spread initial loads -------------
            sl0 = slice(0, 512)
            x_cur = [pp.tile([128, 512], f32r, tag=f"xs{cc}", bufs=2,
                             name=f"xs{cc}") for cc in range(4)]
            # first-wave transfers split across the 3 DMA-capable queues
            # (Pool / SP / ACT) so the first projection matmuls can start
            # ~2.8us in. The ACT queue head carries the hoisted 1.3us
            # activation-table load, so ACT only gets late-needed items.
            nc.gpsimd.dma_start(out=x_cur[0], in_=xT[0:128, sl0])
            nc.gpsimd.dma_start(out=wq_t[1], in_=wqkT[128:256, :])
            nc.gpsimd.dma_start(out=x_cur[3], in_=xT[384:512, sl0])
            nc.gpsimd.dma_start(out=cos_sl[0], in_=cos2T[:, 0:512])
            nc.gpsimd.dma_start(out=wv_t[0], in_=wvT[0:128, :])
            nc.sync.dma_start(out=wq_t[0], in_=wqkT[0:128, :])
            nc.sync.dma_start(out=x_cur[1], in_=xT[128:256, sl0])
            nc.sync.dma_start(out=wq_t[3], in_=wqkT[384:512, :])
            nc.sync.dma_start(out=sin_sl[0], in_=sin2T[:, 0:512])
            nc.sync.dma_start(out=r2_t, in_=r2T)
            nc.sync.dma_start(out=wv_t[1], in_=wvT[128:256, :])
            nc.sync.dma_start(out=wv_t[3], in_=wvT[384:512, :])
            ones_sb = pp.tile([128, 4], f32r, tag="ones_sb")
            nc.sync.dma_start(out=ones_sb, in_=ones4)
            # the ACT queue head carries the hoisted 1.3us activation
            # table load; jump the two critical ACT transfers ahead of it
            # (the first exp isn't needed until ~7us in)
            with tc.high_priority():
                nc.scalar.dma_start(out=wq_t[2], in_=wqkT[256:384, :])
                nc.scalar.dma_start(out=x_cur[2], in_=xT[256:384, sl0])
            # proj(0): only fb0 (q pair0) and fb2 (k pair0) before attention
            # starts -- units (0, hl=0/1) depend just on these + v_aug[0:3].
            # fb1/fb3 of slice 0 flow in as the first iteration-0 fillers.
            p0 = proj_fb_closures(0, 0, x_cur, tag="s", bufs=2,
                                  evac=nc.scalar.copy)
            p2 = proj_fb_closures(0, 2, x_cur, tag="s", bufs=2,
                                  evac=nc.scalar.copy)
            for cl in p0[:4]:
                cl()
            for cl in p2[:4]:
                cl()
            # wv2 rides the ACT queue after the prologue evacuations so it
            # doesn't delay them
            nc.scalar.dma_start(out=wv_t[2], in_=wvT[256:384, :])
            # V-projections interleave with the rope tails so the PE rotate
            # never head-of-line blocks on its evacuation
            p0[4]()   # rope fb0
            emit_vproj(0, 0, x_cur)
            emit_vproj(0, 1, x_cur)
            p2[4]()   # rope fb2
            emit_vproj(0, 2, x_cur)
            emit_vproj(0, 3, x_cur)
            # ones columns of the persistent v_aug tiles, written once,
            # after the slice-0 V copies so those aren't queued behind them
            # (memset doesn't survive neuronxcc codegen: DMA + strided copy)
            for tb in range(nb):
                nc.vector.tensor_copy(
                    out=v_aug[tb].rearrange(
                        "p (h c) -> p h c", h=4)[:, :, 64:65],
                    in_=ones_sb.rearrange("p (h c) -> p h c", c=1))
            prologue_fill = (proj_fb_closures(0, 1, x_cur)
                             + proj_fb_closures(0, 3, x_cur))

            # ---------------- software-pipelined main loop ---------------
            pkq_prev = None
            pkq_by_ts = {}
            x_by_ts = {0: x_cur}
            for ts in range(nt):
                if ts + 1 < nt:
                    x_nxt = load_x(ts + 1, spread=(ts == 0))
                    x_by_ts[ts + 1] = x_nxt
                    if ts == 0:
                        for i in range(2):
                            nc.sync.dma_start(
                                out=wo_t[i], in_=woT[i * 128:(i + 1) * 128, :])
                else:
                    x_nxt = None
                pkq = [pp.tile([128, 512], f32r, tag=f"pkq{i}", bufs=4,
                               name=f"pkq{i}") for i in range(2)]
                pkq_by_ts[ts] = pkq

                # filler queue for this iteration (dependency-safe order).
                # The final iteration's attention alone is exp-bound (the
                # PE would starve behind the ACT exp stream), so every
                # deferrable piece of PE work is parked there: the last
                # slice's V projection and K-pair-1 projection, and the
                # output projections of slices 0 and 2. outproj(1) fills
                # iteration 2, which is roughly exp/PE-balanced.
                def weave(fb_lists, tail_items):
                    # lay out projection fillers so each fb's PE rotate
                    # (which head-of-line blocks the PE queue on its psum
                    # evacuation) lands a full block of other work after
                    # that evacuation was issued
                    out, prev_rest = [], None
                    for L in fb_lists:
                        out.extend(L[:4])
                        if prev_rest is not None:
                            out.append(prev_rest)
                        prev_rest = L[4]
                    extras = list(tail_items)
                    if prev_rest is not None:
                        if extras:
                            out.append(extras.pop(0))
                        out.append(prev_rest)
                    out.extend(extras)
                    return out

                fillers = []
                if x_nxt is not None:
                    nxt = ts + 1
                    last = (nxt == nt - 1)
                    # early iterations evacuate on ACT (exp stream has
                    # slack there and DVE is the scarce engine); late
                    # iterations on DVE (ACT is exp-saturated)
                    ev = nc.scalar.copy if ts < 2 else None
                    fbs = (0, 2, 1) if last else (0, 2, 1, 3)
                    fb_lists = ([] if ts != 0 else
                                [prologue_fill[0:5], prologue_fill[5:10]])
                    fb_lists += [proj_fb_closures(nxt, fb, x_nxt, evac=ev)
                                 for fb in fbs]
                    vps = [] if last else [
                        (lambda nxt=nxt, tbl=tbl, x=x_nxt:
                         emit_vproj(nxt, tbl, x)) for tbl in range(4)]
                    fillers = weave(fb_lists, vps)
                else:
                    # final iteration: V(3) first (needed by the first
                    # unit's trailing diagonal), then K3 pair 1, then the
                    # deferred output projections of slices 1, 0, 2
                    x_fin = x_by_ts[ts]
                    fillers = [
                        (lambda tbl=tbl, x=x_fin: emit_vproj(ts, tbl, x))
                        for tbl in range(4)]
                    ops = []
                    for qs_ in (1, 0, 2):
                        for tbl in range(4):
                            ops.append(
                                lambda qs=qs_, tbl=tbl, p=pkq_by_ts[qs_]:
                                emit_outproj_tbl(qs, tbl, p))
                    fillers += weave([proj_fb_closures(ts, 3, x_fin)], ops)

                # dynamic pacing: spread the fillers evenly over the
                # iteration's pull points so late units still have PE work
                # and early fillers (next slice's rope chains) land well
                # before the next iteration's attention needs them
                n_fill = len(fillers)
                n_pulls = 4 * (2 * ts + 2)
                if ts == nt - 1:
                    # front-load the final iteration's fillers: any excess
                    # PE work woven into the last unit pushes its closing
                    # AVs -- and with them the whole kernel tail -- later
                    n_pulls -= 6
                state = [0, 0]   # executed, pulls seen

                def pull():
                    state[1] += 1
                    target = min(n_fill,
                                 -(-n_fill * state[1] // n_pulls))
                    while state[0] < target:
                        fillers[state[0]]()
                        state[0] += 1

                for hl in range(HL):
                    emit_attn(ts, hl, pkq, pull,
                              diag_first=not (ts == nt - 1 and hl == 0))
                # drain leftover fillers before the next iteration
                while state[0] < n_fill:
                    fillers[state[0]]()
                    state[0] += 1
                pkq_prev = pkq
            for tbl in range(4):
                emit_outproj_tbl(nt - 1, tbl, pkq_prev, tail=True)

    nc.compile()
    return nc


def _rot_mats():
    r = np.zeros((64, 64), np.float32)
    r[np.arange(32), np.arange(32) + 32] = -1.0
    r[np.arange(32, 64), np.arange(32)] = 1.0
    r2 = np.zeros((128, 128), np.float32)
    r2[:64, :64] = r
    r2[64:, 64:] = r
    return np.ascontiguousarray(r2.T)


def _preprocess(x, cos, sin, W_qkv, W_out, t=T):
    r2T = _rot_mats()
    cosT = np.ascontiguousarray(cos[:t].T.astype(np.float32))
    sinT = np.ascontiguousarray(sin[:t].T.astype(np.float32))
    cos2 = np.concatenate([cosT, cosT], 0)
    sin2 = np.concatenate([sinT, sinT], 0)

    in_maps = []
    for c in range(NCORES):
        b, g = divmod(c, 2)
        wq = W_qkv[g * 256:(g + 1) * 256] * 0.125
        wk = W_qkv[512 + g * 256:512 + (g + 1) * 256]
        wv = W_qkv[1024 + g * 256:1024 + (g + 1) * 256]
        in_maps.append({
            "xT": np.ascontiguousarray(x[b, :t].T.astype(np.float32)),
            "wqkT": np.ascontiguousarray(
                np.concatenate([wq, wk], 0).T.astype(np.float32)),
            "wvT": np.ascontiguousarray(wv.T.astype(np.float32)),
            "woT": np.ascontiguousarray(
                W_out.T[g * 256:(g + 1) * 256].astype(np.float32)),
            "cos2T": cos2, "sin2T": sin2, "r2T": r2T,
            "ones4": np.ones((128, 4), np.float32),
        })
    return in_maps


def kernel(x, cos, sin, W_qkv, W_out, _trace=False):
    global LAST_EXEC_NS, LAST_RESULTS
    from concourse.bass_utils import run_bass_kernel_spmd

    x = np.asarray(x); cos = np.asarray(cos); sin = np.asarray(sin)
    W_qkv = np.asarray(W_qkv); W_out = np.asarray(W_out)

    if T not in _prog_cache:
        _prog_cache[T] = _build_program(T)
    nc = _prog_cache[T]

    in_maps = _preprocess(x, cos, sin, W_qkv, W_out)
    try:
        res = run_bass_kernel_spmd(nc, in_maps, list(range(NCORES)),
                                   trace=_trace)
    except ModuleNotFoundError:
        # NTFF profiling hooks unavailable under this axon build
        res = run_bass_kernel_spmd(nc, in_maps, list(range(NCORES)),
                                   trace=False)
    LAST_EXEC_NS = res.exec_time_ns
    LAST_RESULTS = res
    out = np.empty((B, T, C), np.float32)
    for b in range(B):
        out[b] = res.results[2 * b]["y"] + res.results[2 * b + 1]["y"]
    return out
